# revision 52
# baseline (speedup 1.0000x reference)
"""Trainium2 Bass kernel for CrossHeadMultiHeadAttention (v2).

Computation (per batch b, spatial site s): LN over d=64 per head (8 heads),
torch-Linear Q/K/V, cross-head attention within 4 groups of 16 dims,
out-projection, residual.

v2 strategy (data-parallel over batch, 16 -> 8 cores x 2):
  - x stays in d-on-partition layout [(head,d), sites]; NO transposes for LN.
  - LN stats via PE matmuls (ones-moving, N=2); mean-centering enters the
    Q/K/V projections as an extra "-mu" stationary row (mu-row trick).
  - rstd and biases never touch Q/K/V tensors: scores_true factorizes as
      exp(S*scores) = exp(rr_ij * QKc) * [i-terms cancel in softmax]
                      * exp(rstdS_j * kb2)_j
    with rr = rstd_i*rstd_j*SCALE, kb2 = sum_t Kc*bq' (extra matmul cols),
    V-bias folded into the out-projection bias host-side.
  - attention core (per-site 8x8x(4 groups) QK^T / AV) on DVE/Pool as
    broadcast-product tensors + halving-tree reductions, bf16, 2x mode.
  - out-projection via block-diag Wo matmul; residual via identity matmul
    of bf16 x; out bias rides the ACT PSUM->SBUF copy.
"""

import json

import numpy as np
import ml_dtypes

import concourse.bass as bass
import concourse.mybir as mybir
from concourse.tile import TileContext
from concourse.bass_utils import run_bass_kernel_spmd
import concourse.bass_utils as _bass_utils
import concourse.bass2jax as _bass2jax
import bass_rust

F32 = mybir.dt.float32
BF16 = mybir.dt.bfloat16
AX = mybir.AxisListType
OP = mybir.AluOpType
AF = mybir.ActivationFunctionType

N_HEADS = 8
D = 64
A = 4          # attention groups
SD = 16        # sub dim per group
SCALE = SD ** -0.5
LN_EPS = 1e-5
N_CORES = 8

_PATCHED = False

# this walrus build accepts fewer sync-wait commands per instruction than
# bass emits; hoist the excess onto EventSemaphore carriers just before.
_WAIT_CAPS = {"Drain": 0, "Nop": 0, "EventSemaphore": 2}
_DEFAULT_WAIT_CAP = 1


def _fix_bir_waits(bir: bytes) -> bytes:
    j = json.loads(bir)
    ctr = 0
    changed = False
    for f in j.get("functions", []):
        for blk in f.get("blocks", []):
            out = []
            for ins in blk.get("instructions", []):
                si = ins.get("sync_info") or {}
                ow = si.get("on_wait") or []
                cap = _WAIT_CAPS.get(ins.get("opcode"), _DEFAULT_WAIT_CAP)
                if len(ow) > cap:
                    changed = True
                    n_keep = cap
                    excess, keep = ow[: len(ow) - n_keep], ow[len(ow) - n_keep :]
                    for i in range(0, len(excess), 2):
                        ctr += 1
                        chunk = excess[i : i + 2]
                        w0 = chunk[0]
                        out.append({
                            "debug": ins.get("debug", 0),
                            "engine": ins.get("engine"),
                            "ins": [],
                            "outs": [],
                            "name": f"waitfix_{ctr}",
                            "opcode": "EventSemaphore",
                            "sync_info": {
                                "on_update": [{
                                    "ant_name": w0["ant_name"],
                                    "id": w0["id"],
                                    "sync_type": "semaphore",
                                    "update_mode": "sem-add-imm",
                                    "update_value": 0,
                                }],
                                "on_wait": chunk,
                            },
                        })
                    si = dict(si)
                    si["on_wait"] = keep
                    ins = dict(ins)
                    ins["sync_info"] = si
                out.append(ins)
            blk["instructions"] = out
    if not changed:
        return bir
    return json.dumps(j).encode()


_orig_compile_bir_kernel = _bass_utils.compile_bir_kernel


def _compile_bir_kernel_fixed(bir_json, tmpdir, neff_name="file.neff"):
    if isinstance(bir_json, str):
        bir_json = bir_json.encode()
    return _orig_compile_bir_kernel(_fix_bir_waits(bir_json), tmpdir, neff_name=neff_name)


def _patch_tile_drain():
    """walrus here rejects >2 sem waits on the Tile tail-drain; spread the
    waits over EventSemaphore carriers (<=2 waits each) instead."""
    global _PATCHED
    if _PATCHED:
        return
    _PATCHED = True
    _bass_utils.compile_bir_kernel = _compile_bir_kernel_fixed
    _bass2jax.compile_bir_kernel = _compile_bir_kernel_fixed
    ScopedClock = bass_rust.ScopedClock

    def patched(self, tick_clock, wait_clock):
        nc = self.nc
        sems = list(self.sems.allocated().values())
        if sems:
            carrier = nc.sync.sem_inc(sems[0], 0)
            wait_clock.add_sem_waits(
                carrier.ins, ScopedClock({None: tick_clock.global_clock})
            )
            si = carrier.ins.sync_info
            waits = list(si.on_wait) if si else []
            if len(waits) > 2:
                carrier.ins.sync_info = bass_rust.SyncInfo(
                    on_wait=waits[:2], on_update=list(si.on_update)
                )
                for i in range(2, len(waits), 2):
                    c2 = nc.sync.sem_inc(sems[0], 0)
                    si2 = c2.ins.sync_info
                    c2.ins.sync_info = bass_rust.SyncInfo(
                        on_wait=waits[i : i + 2],
                        on_update=list(si2.on_update) if si2 else [],
                    )
        nc.sync.drain()
        nc.all_engine_barrier()
        popped = nc._tile_sem_poison_stack.pop()
        assert popped is self._sem_poison
        nc.clear_and_free_semaphores(sems)
        nc.all_engine_barrier()

    TileContext._drain_and_barrier = patched


def build_nc(n_b: int, s_total: int, st_sites: int):
    """Build the per-core SPMD program.

    n_b: batches per core; s_total: sites per batch (H*W);
    st_sites: sites per super-tile (DMA granularity), multiple of 128.
    """
    _patch_tile_drain()
    nc = bass.Bass()
    TILE = 128
    n_st = s_total // st_sites
    n_t = st_sites // TILE
    NC4 = 4  # head-pair chunks

    x_d = nc.dram_tensor("x", [n_b, N_HEADS, D, s_total], F32, kind="ExternalInput")
    wq_d = nc.dram_tensor("wq_bd", [128, 128], BF16, kind="ExternalInput")
    wk_d = nc.dram_tensor("wk_bd", [128, 128], BF16, kind="ExternalInput")
    wv_d = nc.dram_tensor("wv_bd", [128, 128], BF16, kind="ExternalInput")
    wo_d = nc.dram_tensor("wo_bd", [128, 128], BF16, kind="ExternalInput")
    wkf_d = nc.dram_tensor("wkf_bd", [128, 8], BF16, kind="ExternalInput")
    wsq_d = nc.dram_tensor("wsum_q", [8, 512], BF16, kind="ExternalInput")
    wsk_d = nc.dram_tensor("wsum_k", [8, 512], BF16, kind="ExternalInput")
    wsv_d = nc.dram_tensor("wsum_v", [8, 512], BF16, kind="ExternalInput")
    kbmu_d = nc.dram_tensor("kbmu", [8, 32], BF16, kind="ExternalInput")
    ones2_d = nc.dram_tensor("ones2", [128, 2], BF16, kind="ExternalInput")
    idb_d = nc.dram_tensor("ident_bf", [128, 128], BF16, kind="ExternalInput")
    bo_d = nc.dram_tensor("bo_col", [128, 1], F32, kind="ExternalInput")
    eps_d = nc.dram_tensor("eps_col", [128, 1], F32, kind="ExternalInput")
    out_d = nc.dram_tensor("out", [n_b, N_HEADS, D, s_total], F32, kind="ExternalOutput")

    with TileContext(nc) as tc:
        with (
            tc.tile_pool(name="consts", bufs=1) as cpool,
            tc.tile_pool(name="xio", bufs=2) as xpool,
            tc.tile_pool(name="xbfp", bufs=2) as bpool,
            tc.tile_pool(name="oio", bufs=2) as opool,
            tc.tile_pool(name="work", bufs=4) as wpool,
            tc.tile_pool(name="vecs", bufs=4) as vpool,
            tc.tile_pool(name="stats", bufs=2) as spool,
            tc.tile_pool(name="psst", bufs=1, space="PSUM") as stpool,
            tc.tile_pool(name="psqkv", bufs=1, space="PSUM") as qkvpool,
            tc.tile_pool(name="pso", bufs=1, space="PSUM") as oppool,
        ):
            # ---- constants into SBUF
            def cload(dram, shape, dtype, tag, rows=None):
                t = cpool.tile(shape, dtype, tag=tag)
                if rows is None:
                    nc.sync.dma_start(out=t[:], in_=dram[:])
                else:
                    nc.sync.dma_start(out=t[0:rows, :], in_=dram[:])
                return t

            wq = cload(wq_d, [128, 128], BF16, "wq")
            wk = cload(wk_d, [128, 128], BF16, "wk")
            wv = cload(wv_d, [128, 128], BF16, "wv")
            wo = cload(wo_d, [128, 128], BF16, "wo")
            wkf = cload(wkf_d, [128, 8], BF16, "wkf")
            wsq = cload(wsq_d, [128, 512], BF16, "wsq", rows=8)
            wsk = cload(wsk_d, [128, 512], BF16, "wsk", rows=8)
            wsv = cload(wsv_d, [128, 512], BF16, "wsv", rows=8)
            kbmu = cload(kbmu_d, [128, 32], BF16, "kbmu", rows=8)
            ones2 = cload(ones2_d, [128, 2], BF16, "ones2")
            idb = cload(idb_d, [128, 128], BF16, "idb")
            bo = cload(bo_d, [128, 1], F32, "bo")
            eps = cload(eps_d, [128, 1], F32, "eps")

            for b in range(n_b):
                for st in range(n_st):
                    # ---- load super-tile: 4 chunks of [128=(2n,64d), st_sites]
                    x_sb = xpool.tile([128, NC4 * st_sites], F32, tag="x_sb")
                    xv = x_d[b].rearrange("n d s -> (n d) s")
                    for c in range(NC4):
                        nc.sync.dma_start(
                            out=x_sb[:, c * st_sites : (c + 1) * st_sites],
                            in_=xv[c * 128 : (c + 1) * 128,
                                   st * st_sites : (st + 1) * st_sites],
                        )
                    # bf16 conversion (per chunk, ACT) and squares (DVE 2x)
                    xbf = bpool.tile([128, NC4 * st_sites], BF16, tag="xbf")
                    for c in range(NC4):
                        nc.scalar.copy(
                            xbf[:, c * st_sites : (c + 1) * st_sites],
                            x_sb[:, c * st_sites : (c + 1) * st_sites],
                        )
                    xsq = bpool.tile([128, NC4 * st_sites], BF16, tag="xsq")
                    for c in range(NC4):
                        nc.scalar.activation(
                            xsq[:, c * st_sites : (c + 1) * st_sites],
                            xbf[:, c * st_sites : (c + 1) * st_sites],
                            AF.Square,
                        )
                    out_sb = opool.tile([128, NC4 * st_sites], F32, tag="out_sb")

                    # ---- per-ST stats: 8 matmuls per tile into ps_st
                    # col layout per tile k: [k*16 + (c*2 + h2)] sums,
                    #                        [k*16 + 8 + (c*2 + h2)] sumsq
                    ps_st = stpool.tile([128, n_t * 16], F32, tag="ps_st")
                    for k in range(n_t):
                        for c in range(NC4):
                            nc.tensor.matmul(
                                ps_st[:, k * 16 + c * 2 : k * 16 + c * 2 + 2],
                                xbf[:, c * st_sites + k * TILE :
                                       c * st_sites + (k + 1) * TILE],
                                ones2[:],
                                start=True, stop=True,
                            )
                            nc.tensor.matmul(
                                ps_st[:, k * 16 + 8 + c * 2 : k * 16 + 8 + c * 2 + 2],
                                xsq[:, c * st_sites + k * TILE :
                                       c * st_sites + (k + 1) * TILE],
                                ones2[:],
                                start=True, stop=True,
                            )
                    # ---- batched stat math over [128, (k, 8)]
                    nst = n_t * 8
                    sview = ps_st[:, 0 : n_t * 16].rearrange(
                        "p (k two h) -> p k two h", k=n_t, two=2
                    )
                    mun = spool.tile([128, nst], BF16, tag="mun")      # -mu
                    musq = spool.tile([128, nst], F32, tag="musq")
                    var = spool.tile([128, nst], F32, tag="var")
                    rstd = spool.tile([128, nst], BF16, tag="rstd")
                    rstdS = spool.tile([128, nst], BF16, tag="rstdS")
                    rr = spool.tile([128, n_t * 64], BF16, tag="rr")
                    mun3 = mun[:].rearrange("p (k h) -> p k h", k=n_t)
                    nc.vector.tensor_scalar(
                        mun3, sview[:, :, 0], -1.0 / 64.0, None, op0=OP.mult
                    )
                    nc.vector.tensor_tensor(
                        musq[:].rearrange("p (k h) -> p k h", k=n_t),
                        mun3, mun3, op=OP.mult,
                    )
                    nc.vector.scalar_tensor_tensor(
                        var[:].rearrange("p (k h) -> p k h", k=n_t),
                        sview[:, :, 1], 1.0 / 64.0,
                        musq[:].rearrange("p (k h) -> p k h", k=n_t),
                        op0=OP.mult, op1=OP.subtract,
                    )
                    nc.scalar.activation(var[:], var[:], AF.Sqrt, bias=eps[:, 0:1])
                    with nc.allow_low_precision(reason="rstd in bf16"):
                        nc.vector.reciprocal(rstd[:], var[:])
                    nc.vector.tensor_scalar(rstdS[:], rstd[:], SCALE, None, op0=OP.mult)
                    # rr[p, k, i, j] = rstd_i * rstdS_j
                    nc.vector.tensor_tensor(
                        rr[:].rearrange("p (k i j) -> p k i j", k=n_t, i=8),
                        rstd[:].rearrange("p (k i) -> p k i", k=n_t)
                            .unsqueeze(3).broadcast_to([128, n_t, 8, 8]),
                        rstdS[:].rearrange("p (k j) -> p k j", k=n_t)
                            .unsqueeze(2).broadcast_to([128, n_t, 8, 8]),
                        op=OP.mult,
                    )
                    rstd3 = rstd[:].rearrange("p (k h) -> p k h", k=n_t)
                    rstdS3 = rstdS[:].rearrange("p (k h) -> p k h", k=n_t)
                    rr3 = rr[:].rearrange("p (k f) -> p k f", k=n_t)

                    # ---- phase B: per tile
                    for k in range(n_t):
                        # munT: [128,8] -> [8,128] via PE transpose
                        ps_mt = stpool.tile([128, 128], BF16, tag="ps_mt")
                        nc.tensor.transpose(
                            ps_mt[0:8, :], mun[:, k * 8 : (k + 1) * 8], idb[:]
                        )
                        mun_sb = vpool.tile([128, 128], BF16, tag="mun_sb")
                        nc.scalar.copy(mun_sb[0:8, :], ps_mt[0:8, :])

                        def xslice(c):
                            return xbf[:, c * st_sites + k * TILE :
                                          c * st_sites + (k + 1) * TILE]

                        # ---- projections: mu-row first (start=True), 4 chunks
                        ps_q = qkvpool.tile([128, 512], F32, tag="ps_q")
                        ps_k = qkvpool.tile([128, 512], F32, tag="ps_k")
                        ps_v = qkvpool.tile([128, 512], F32, tag="ps_v")
                        ps_kb = qkvpool.tile([128, 32], F32, tag="ps_kb")
                        for ps_p, ws_p in ((ps_q, wsq), (ps_k, wsk), (ps_v, wsv)):
                            nc.tensor.matmul(
                                ps_p[:], mun_sb[0:8, :], ws_p[0:8, :],
                                start=True, stop=False,
                            )
                        nc.tensor.matmul(
                            ps_kb[:, 0:32], mun_sb[0:8, :], kbmu[0:8, :],
                            start=True, stop=False,
                        )
                        for c in range(NC4):
                            xs = xslice(c)
                            for ps_p, w_p in ((ps_q, wq), (ps_k, wk), (ps_v, wv)):
                                nc.tensor.matmul(
                                    ps_p[:, c * 128 : (c + 1) * 128],
                                    xs, w_p[:],
                                    start=False, stop=(c == NC4 - 1),
                                )
                            nc.tensor.matmul(
                                ps_kb[:, c * 8 : (c + 1) * 8],
                                xs, wkf[:],
                                start=False, stop=(c == NC4 - 1),
                            )

                        # ---- PSUM -> SBUF copies
                        q_sb = vpool.tile([128, 512], BF16, tag="q_sb")
                        k_sb = vpool.tile([128, 512], BF16, tag="k_sb")
                        v_sb = vpool.tile([128, 512], BF16, tag="v_sb")
                        nc.scalar.copy(q_sb[:], ps_q[:])
                        nc.scalar.copy(k_sb[:], ps_k[:])
                        # V stored (a, t, j) so AV products get unit inner strides
                        nc.scalar.activation(
                            v_sb[:].rearrange("p (a t j) -> p a j t", a=A, t=SD, j=8),
                            ps_v[:].rearrange("p (j a t) -> p a j t", j=8, a=A, t=SD),
                            AF.Copy,
                        )
                        # T3[p,(a,j)] = kb2 * rstdS_j ; E3 = exp(T3)
                        t3 = wpool.tile([128, 32], BF16, tag="t3")
                        t3v = t3[:].rearrange("p (a j) -> p a j", a=A)
                        nc.vector.tensor_tensor(
                            t3v,
                            ps_kb[:, 0:32].rearrange("p (j a) -> p a j", j=8),
                            rstdS3[:, k].unsqueeze(1).broadcast_to([128, A, 8]),
                            op=OP.mult,
                        )
                        e3f = wpool.tile([128, 32], BF16, tag="e3f")
                        nc.scalar.activation(e3f[:], t3[:], AF.Exp)

                        # ---- scores products [128, (a,i,j,t)] — split per a
                        prod_s = wpool.tile([128, 4096], BF16, tag="big")
                        qv = q_sb[:].rearrange("p (i a t) -> p a i t", i=8, a=A, t=SD)
                        kv = k_sb[:].rearrange("p (j a t) -> p a j t", j=8, a=A, t=SD)
                        pv5 = prod_s[:].rearrange(
                            "p (a i j t) -> p a i j t", a=A, i=8, j=8, t=SD
                        )
                        for a in range(A):
                            eng_s = nc.gpsimd if a == 3 else nc.vector
                            eng_s.tensor_tensor(
                                pv5[:, a],
                                qv[:, a].unsqueeze(2).broadcast_to([128, 8, 8, SD]),
                                kv[:, a].unsqueeze(1).broadcast_to([128, 8, 8, SD]),
                                op=OP.mult,
                            )
                        # t-tree: 16 -> 8 -> 4 -> 2 -> 1
                        t8 = wpool.tile([128, 2048], BF16, tag="t4k")
                        pv = prod_s[:].rearrange("p (r t) -> p r t", r=256, t=SD)
                        nc.vector.tensor_tensor(
                            t8[:].rearrange("p (r t) -> p r t", r=256, t=8),
                            pv[:, :, 0:8], pv[:, :, 8:16], op=OP.add,
                        )
                        t4 = wpool.tile([128, 1024], BF16, tag="t2k")
                        t8v = t8[:].rearrange("p (r t) -> p r t", r=256, t=8)
                        nc.vector.tensor_tensor(
                            t4[:].rearrange("p (r t) -> p r t", r=256, t=4),
                            t8v[:, :, 0:4], t8v[:, :, 4:8], op=OP.add,
                        )
                        t2 = wpool.tile([128, 512], BF16, tag="t1k")
                        t4v = t4[:].rearrange("p (r t) -> p r t", r=256, t=4)
                        nc.vector.tensor_tensor(
                            t2[:].rearrange("p (r t) -> p r t", r=256, t=2),
                            t4v[:, :, 0:2], t4v[:, :, 2:4], op=OP.add,
                        )
                        sraw = wpool.tile([128, 256], BF16, tag="sraw")
                        t2v = t2[:].rearrange("p (r t) -> p r t", r=256, t=2)
                        nc.vector.tensor_tensor(
                            sraw[:].rearrange("p (r t) -> p r t", r=256, t=1),
                            t2v[:, :, 0:1], t2v[:, :, 1:2], op=OP.add,
                        )
                        # e_in = sraw * rr (bcast over a) ; e = exp(e_in)
                        e_in = wpool.tile([128, 256], BF16, tag="e_in")
                        nc.vector.tensor_tensor(
                            e_in[:].rearrange("p (a i j) -> p a i j", a=A, i=8),
                            sraw[:].rearrange("p (a i j) -> p a i j", a=A, i=8),
                            rr3[:, k].rearrange("p (i j) -> p i j", i=8)
                                .unsqueeze(1).broadcast_to([128, A, 8, 8]),
                            op=OP.mult,
                        )
                        e = wpool.tile([128, 256], BF16, tag="e")
                        with tc.high_priority(offset=40):
                            nc.scalar.activation(e[:], e_in[:], AF.Exp)
                        # e2 = e * E3_j (bcast over i)
                        e2 = wpool.tile([128, 256], BF16, tag="e2")
                        nc.vector.tensor_tensor(
                            e2[:].rearrange("p (a i j) -> p a i j", a=A, i=8),
                            e[:].rearrange("p (a i j) -> p a i j", a=A, i=8),
                            e3f[:].rearrange("p (a j) -> p a j", a=A)
                                .unsqueeze(2).broadcast_to([128, A, 8, 8]),
                            op=OP.mult,
                        )
                        den = wpool.tile([128, 32], F32, tag="den")
                        nc.vector.tensor_reduce(
                            den[:],
                            e2[:].rearrange("p (r j) -> p r j", r=32, j=8),
                            axis=AX.X, op=OP.add,
                        )
                        rcp = wpool.tile([128, 32], BF16, tag="rcp")
                        with nc.allow_low_precision(reason="softmax denom in bf16"):
                            nc.vector.reciprocal(rcp[:], den[:])
                        # w = rcp_(a,i) * rstd_j ; e3 = e2 * w
                        wgt = wpool.tile([128, 256], BF16, tag="wgt")
                        nc.vector.tensor_tensor(
                            wgt[:].rearrange("p (a i j) -> p a i j", a=A, i=8),
                            rcp[:].rearrange("p (a i) -> p a i", a=A)
                                .unsqueeze(3).broadcast_to([128, A, 8, 8]),
                            rstd3[:, k].unsqueeze(1).unsqueeze(1)
                                .broadcast_to([128, A, 8, 8]),
                            op=OP.mult,
                        )
                        e3 = wpool.tile([128, 256], BF16, tag="e3")
                        nc.vector.tensor_tensor(e3[:], e2[:], wgt[:], op=OP.mult)

                        # ---- AV products [128, (a,i,t,j)] — split per a
                        prod_av = wpool.tile([128, 4096], BF16, tag="big")
                        ev = e3[:].rearrange("p (a i j) -> p a i j", a=A, i=8)
                        vv = v_sb[:].rearrange("p (a t j) -> p a t j", a=A, t=SD, j=8)
                        av5 = prod_av[:].rearrange(
                            "p (a i t j) -> p a i t j", a=A, i=8, t=SD, j=8
                        )
                        for a in range(A):
                            eng = nc.vector if a <= 2 else nc.gpsimd
                            eng.tensor_tensor(
                                av5[:, a],
                                ev[:, a].unsqueeze(2).broadcast_to([128, 8, SD, 8]),
                                vv[:, a].unsqueeze(1).broadcast_to([128, 8, SD, 8]),
                                op=OP.mult,
                            )
                        # j-tree: 8 -> 4 -> 2 -> 1
                        j4 = wpool.tile([128, 2048], BF16, tag="t4k")
                        av = prod_av[:].rearrange("p (r j) -> p r j", r=512, j=8)
                        nc.vector.tensor_tensor(
                            j4[:].rearrange("p (r j) -> p r j", r=512, j=4),
                            av[:, :, 0:4], av[:, :, 4:8], op=OP.add,
                        )
                        j2 = wpool.tile([128, 1024], BF16, tag="t2k")
                        j4v = j4[:].rearrange("p (r j) -> p r j", r=512, j=4)
                        nc.vector.tensor_tensor(
                            j2[:].rearrange("p (r j) -> p r j", r=512, j=2),
                            j4v[:, :, 0:2], j4v[:, :, 2:4], op=OP.add,
                        )
                        # final level writes (i,a,t)-ordered attnout
                        attnout = vpool.tile([128, 512], BF16, tag="attnout")
                        j2v = j2[:].rearrange(
                            "p (a i t j) -> p a i t j", a=A, i=8, t=SD, j=2
                        )
                        nc.vector.tensor_tensor(
                            attnout[:].rearrange(
                                "p (i a t) -> p a i t", i=8, a=A, t=SD
                            ),
                            j2v[:, :, :, :, 0],
                            j2v[:, :, :, :, 1],
                            op=OP.add,
                        )
                        # ---- out projection
                        ps_aoT = oppool.tile([128, 512], BF16, tag="ps_aoT")
                        for c in range(NC4):
                            nc.tensor.transpose(
                                ps_aoT[:, c * 128 : (c + 1) * 128],
                                attnout[:, c * 128 : (c + 1) * 128],
                                idb[:],
                            )
                        ao_sb = vpool.tile([128, 512], BF16, tag="ao_sb")
                        nc.scalar.copy(ao_sb[:], ps_aoT[:])
                        ps_o = oppool.tile([128, 512], F32, tag="ps_o")
                        # residual first (c0 start=True clears bank)
                        for c in range(NC4):
                            nc.tensor.matmul(
                                ps_o[:, c * 128 : (c + 1) * 128],
                                idb[:], xslice(c),
                                start=(c == 0), stop=False,
                            )
                        for c in range(NC4):
                            nc.tensor.matmul(
                                ps_o[:, c * 128 : (c + 1) * 128],
                                wo[:],
                                ao_sb[:, c * 128 : (c + 1) * 128],
                                start=False, stop=(c == NC4 - 1),
                            )
                        # + bo, PSUM -> out_sb
                        ov = out_sb[:].rearrange(
                            "p (c s) -> p c s", c=NC4, s=st_sites
                        )[:, :, k * TILE : (k + 1) * TILE]
                        nc.scalar.activation(
                            ov,
                            ps_o[:].rearrange("p (c s) -> p c s", c=NC4, s=TILE),
                            AF.Identity,
                            bias=bo[:, 0:1],
                        )
                    # ---- store super-tile
                    ovd = out_d[b].rearrange("n d s -> (n d) s")
                    for c in range(NC4):
                        nc.sync.dma_start(
                            out=ovd[c * 128 : (c + 1) * 128,
                                    st * st_sites : (st + 1) * st_sites],
                            in_=out_sb[:, c * st_sites : (c + 1) * st_sites],
                        )
    return nc


def _prep_consts(Wq, bq, Wk, bk, Wv, bv, Wo, bo, ln_g, ln_b):
    f32 = np.float32
    bf = ml_dtypes.bfloat16
    Wq, bq, Wk, bk, Wv, bv, Wo, bo, ln_g, ln_b = [
        np.asarray(t, f32) for t in (Wq, bq, Wk, bk, Wv, bv, Wo, bo, ln_g, ln_b)
    ]
    # fold LN gain into projections; LN beta into per-head biases
    Wq_p, Wk_p, Wv_p = Wq * ln_g[None, :], Wk * ln_g[None, :], Wv * ln_g[None, :]
    BQ = bq + Wq @ ln_b
    BK = bk + Wk @ ln_b  # noqa: F841  (enters only via cc, which cancels)
    BV = bv + Wv @ ln_b
    # kb-fold: kb2[s,(j,a)] = xc_j . wkf[:,a], wkf[d,a] = sum_t Wk_p[(a,t),d]*BQ[(a,t)]
    wkf = np.zeros((64, A), f32)
    for a in range(A):
        for t in range(SD):
            wkf[:, a] += Wk_p[a * SD + t, :] * BQ[a * SD + t]
    bo_p = bo + Wo @ BV  # V-bias folded through out-projection

    def blockdiag(W):
        m = np.zeros((128, 128), f32)
        m[:64, :64] = W.T
        m[64:, 64:] = W.T
        return m.astype(bf)

    wkf_bd = np.zeros((128, 8), f32)
    wkf_bd[:64, 0:4] = wkf
    wkf_bd[64:, 4:8] = wkf

    # mu-row movings: row h, col (h',o): delta * wsum[o]
    def wsum_rows(Wp):
        ws = Wp.sum(axis=1)  # [64] per output col o
        m = np.zeros((8, 512), f32)
        for h in range(8):
            m[h, h * 64 : (h + 1) * 64] = ws
        return m.astype(bf)

    kbmu = np.zeros((8, 32), f32)
    wkfsum = wkf.sum(axis=0)  # [A]
    for j in range(8):
        kbmu[j, j * 4 : (j + 1) * 4] = wkfsum

    ones2 = np.zeros((128, 2), f32)
    ones2[:64, 0] = 1.0
    ones2[64:, 1] = 1.0

    consts = {
        "wq_bd": blockdiag(Wq_p),
        "wk_bd": blockdiag(Wk_p),
        "wv_bd": blockdiag(Wv_p),
        "wo_bd": blockdiag(Wo),
        "wkf_bd": wkf_bd.astype(bf),
        "wsum_q": wsum_rows(Wq_p),
        "wsum_k": wsum_rows(Wk_p),
        "wsum_v": wsum_rows(Wv_p),
        "kbmu": kbmu.astype(bf),
        "ones2": ones2.astype(bf),
        "ident_bf": np.eye(128).astype(bf),
        "bo_col": np.tile(bo_p, 2)[:, None].astype(f32),
        "eps_col": np.full((128, 1), LN_EPS, f32),
    }
    return consts


def kernel(x, Wq, bq, Wk, bk, Wv, bv, Wo, bo, ln_g, ln_b):
    x = np.asarray(x, np.float32)
    B, n, d, H, W = x.shape
    S = H * W
    bpc = B // N_CORES
    consts = _prep_consts(Wq, bq, Wk, bk, Wv, bv, Wo, bo, ln_g, ln_b)

    nc = build_nc(n_b=bpc, s_total=S, st_sites=1024 if S % 1024 == 0 else S)
    xr = x.reshape(B, n, d, S)
    in_maps = []
    for c in range(N_CORES):
        m = dict(consts)
        m["x"] = np.ascontiguousarray(xr[c * bpc : (c + 1) * bpc])
        in_maps.append(m)
    res = run_bass_kernel_spmd(nc, in_maps, core_ids=list(range(N_CORES)))
    outs = [res.results[i]["out"] for i in range(N_CORES)]
    out = np.concatenate(outs, axis=0).reshape(B, n, d, H, W)
    return out.astype(np.float32)


# revision 54
# speedup vs baseline: 1.0025x; 1.0025x over previous
"""Trainium2 Bass kernel for CrossHeadMultiHeadAttention (v2).

Computation (per batch b, spatial site s): LN over d=64 per head (8 heads),
torch-Linear Q/K/V, cross-head attention within 4 groups of 16 dims,
out-projection, residual.

v2 strategy (data-parallel over batch, 16 -> 8 cores x 2):
  - x stays in d-on-partition layout [(head,d), sites]; NO transposes for LN.
  - LN stats via PE matmuls (ones-moving, N=2); mean-centering enters the
    Q/K/V projections as an extra "-mu" stationary row (mu-row trick).
  - rstd and biases never touch Q/K/V tensors: scores_true factorizes as
      exp(S*scores) = exp(rr_ij * QKc) * [i-terms cancel in softmax]
                      * exp(rstdS_j * kb2)_j
    with rr = rstd_i*rstd_j*SCALE, kb2 = sum_t Kc*bq' (extra matmul cols),
    V-bias folded into the out-projection bias host-side.
  - attention core (per-site 8x8x(4 groups) QK^T / AV) on DVE/Pool as
    broadcast-product tensors + halving-tree reductions, bf16, 2x mode.
  - out-projection via block-diag Wo matmul; residual via identity matmul
    of bf16 x; out bias rides the ACT PSUM->SBUF copy.
"""

import json

import numpy as np
import ml_dtypes

import concourse.bass as bass
import concourse.mybir as mybir
from concourse.tile import TileContext
from concourse.bass_utils import run_bass_kernel_spmd
import concourse.bass_utils as _bass_utils
import concourse.bass2jax as _bass2jax
import bass_rust

F32 = mybir.dt.float32
BF16 = mybir.dt.bfloat16
AX = mybir.AxisListType
OP = mybir.AluOpType
AF = mybir.ActivationFunctionType

N_HEADS = 8
D = 64
A = 4          # attention groups
SD = 16        # sub dim per group
SCALE = SD ** -0.5
LN_EPS = 1e-5
N_CORES = 8

_PATCHED = False

# this walrus build accepts fewer sync-wait commands per instruction than
# bass emits; hoist the excess onto EventSemaphore carriers just before.
_WAIT_CAPS = {"Drain": 0, "Nop": 0, "EventSemaphore": 2}
_DEFAULT_WAIT_CAP = 1


def _fix_bir_waits(bir: bytes) -> bytes:
    j = json.loads(bir)
    ctr = 0
    changed = False
    for f in j.get("functions", []):
        for blk in f.get("blocks", []):
            out = []
            for ins in blk.get("instructions", []):
                si = ins.get("sync_info") or {}
                ow = si.get("on_wait") or []
                cap = _WAIT_CAPS.get(ins.get("opcode"), _DEFAULT_WAIT_CAP)
                if len(ow) > cap:
                    changed = True
                    n_keep = cap
                    excess, keep = ow[: len(ow) - n_keep], ow[len(ow) - n_keep :]
                    for i in range(0, len(excess), 2):
                        ctr += 1
                        chunk = excess[i : i + 2]
                        w0 = chunk[0]
                        out.append({
                            "debug": ins.get("debug", 0),
                            "engine": ins.get("engine"),
                            "ins": [],
                            "outs": [],
                            "name": f"waitfix_{ctr}",
                            "opcode": "EventSemaphore",
                            "sync_info": {
                                "on_update": [{
                                    "ant_name": w0["ant_name"],
                                    "id": w0["id"],
                                    "sync_type": "semaphore",
                                    "update_mode": "sem-add-imm",
                                    "update_value": 0,
                                }],
                                "on_wait": chunk,
                            },
                        })
                    si = dict(si)
                    si["on_wait"] = keep
                    ins = dict(ins)
                    ins["sync_info"] = si
                out.append(ins)
            blk["instructions"] = out
    if not changed:
        return bir
    return json.dumps(j).encode()


_orig_compile_bir_kernel = _bass_utils.compile_bir_kernel


def _compile_bir_kernel_fixed(bir_json, tmpdir, neff_name="file.neff"):
    if isinstance(bir_json, str):
        bir_json = bir_json.encode()
    return _orig_compile_bir_kernel(_fix_bir_waits(bir_json), tmpdir, neff_name=neff_name)


def _patch_tile_drain():
    """walrus here rejects >2 sem waits on the Tile tail-drain; spread the
    waits over EventSemaphore carriers (<=2 waits each) instead."""
    global _PATCHED
    if _PATCHED:
        return
    _PATCHED = True
    _bass_utils.compile_bir_kernel = _compile_bir_kernel_fixed
    _bass2jax.compile_bir_kernel = _compile_bir_kernel_fixed
    ScopedClock = bass_rust.ScopedClock

    def patched(self, tick_clock, wait_clock):
        nc = self.nc
        sems = list(self.sems.allocated().values())
        if sems:
            carrier = nc.sync.sem_inc(sems[0], 0)
            wait_clock.add_sem_waits(
                carrier.ins, ScopedClock({None: tick_clock.global_clock})
            )
            si = carrier.ins.sync_info
            waits = list(si.on_wait) if si else []
            if len(waits) > 2:
                carrier.ins.sync_info = bass_rust.SyncInfo(
                    on_wait=waits[:2], on_update=list(si.on_update)
                )
                for i in range(2, len(waits), 2):
                    c2 = nc.sync.sem_inc(sems[0], 0)
                    si2 = c2.ins.sync_info
                    c2.ins.sync_info = bass_rust.SyncInfo(
                        on_wait=waits[i : i + 2],
                        on_update=list(si2.on_update) if si2 else [],
                    )
        nc.sync.drain()
        nc.all_engine_barrier()
        popped = nc._tile_sem_poison_stack.pop()
        assert popped is self._sem_poison
        nc.clear_and_free_semaphores(sems)
        nc.all_engine_barrier()

    TileContext._drain_and_barrier = patched


def build_nc(n_b: int, s_total: int, st_sites: int):
    """Build the per-core SPMD program.

    n_b: batches per core; s_total: sites per batch (H*W);
    st_sites: sites per super-tile (DMA granularity), multiple of 128.
    """
    _patch_tile_drain()
    nc = bass.Bass()
    TILE = 128
    n_st = s_total // st_sites
    n_t = st_sites // TILE
    NC4 = 4  # head-pair chunks

    x_d = nc.dram_tensor("x", [n_b, N_HEADS, D, s_total], F32, kind="ExternalInput")
    wq_d = nc.dram_tensor("wq_bd", [128, 128], BF16, kind="ExternalInput")
    wk_d = nc.dram_tensor("wk_bd", [128, 128], BF16, kind="ExternalInput")
    wv_d = nc.dram_tensor("wv_bd", [128, 128], BF16, kind="ExternalInput")
    wo_d = nc.dram_tensor("wo_bd", [128, 128], BF16, kind="ExternalInput")
    wkf_d = nc.dram_tensor("wkf_bd", [128, 8], BF16, kind="ExternalInput")
    wsq_d = nc.dram_tensor("wsum_q", [8, 512], BF16, kind="ExternalInput")
    wsk_d = nc.dram_tensor("wsum_k", [8, 512], BF16, kind="ExternalInput")
    wsv_d = nc.dram_tensor("wsum_v", [8, 512], BF16, kind="ExternalInput")
    kbmu_d = nc.dram_tensor("kbmu", [8, 32], BF16, kind="ExternalInput")
    ones2_d = nc.dram_tensor("ones2", [128, 2], BF16, kind="ExternalInput")
    idb_d = nc.dram_tensor("ident_bf", [128, 128], BF16, kind="ExternalInput")
    bo_d = nc.dram_tensor("bo_col", [128, 1], F32, kind="ExternalInput")
    eps_d = nc.dram_tensor("eps_col", [128, 1], F32, kind="ExternalInput")
    out_d = nc.dram_tensor("out", [n_b, N_HEADS, D, s_total], F32, kind="ExternalOutput")

    with TileContext(nc) as tc:
        with (
            tc.tile_pool(name="consts", bufs=1) as cpool,
            tc.tile_pool(name="xio", bufs=2) as xpool,
            tc.tile_pool(name="xbfp", bufs=2) as bpool,
            tc.tile_pool(name="oio", bufs=2) as opool,
            tc.tile_pool(name="work", bufs=4) as wpool,
            tc.tile_pool(name="vecs", bufs=4) as vpool,
            tc.tile_pool(name="stats", bufs=2) as spool,
            tc.tile_pool(name="psst", bufs=1, space="PSUM") as stpool,
            tc.tile_pool(name="psqkv", bufs=1, space="PSUM") as qkvpool,
            tc.tile_pool(name="pso", bufs=1, space="PSUM") as oppool,
        ):
            # ---- constants into SBUF
            def cload(dram, shape, dtype, tag, rows=None):
                t = cpool.tile(shape, dtype, tag=tag)
                if rows is None:
                    nc.sync.dma_start(out=t[:], in_=dram[:])
                else:
                    nc.sync.dma_start(out=t[0:rows, :], in_=dram[:])
                return t

            wq = cload(wq_d, [128, 128], BF16, "wq")
            wk = cload(wk_d, [128, 128], BF16, "wk")
            wv = cload(wv_d, [128, 128], BF16, "wv")
            wo = cload(wo_d, [128, 128], BF16, "wo")
            wkf = cload(wkf_d, [128, 8], BF16, "wkf")
            wsq = cload(wsq_d, [128, 512], BF16, "wsq", rows=8)
            wsk = cload(wsk_d, [128, 512], BF16, "wsk", rows=8)
            wsv = cload(wsv_d, [128, 512], BF16, "wsv", rows=8)
            kbmu = cload(kbmu_d, [128, 32], BF16, "kbmu", rows=8)
            ones2 = cload(ones2_d, [128, 2], BF16, "ones2")
            idb = cload(idb_d, [128, 128], BF16, "idb")
            bo = cload(bo_d, [128, 1], F32, "bo")
            eps = cload(eps_d, [128, 1], F32, "eps")

            for b in range(n_b):
                for st in range(n_st):
                    # ---- load super-tile: 4 chunks of [128=(2n,64d), st_sites]
                    x_sb = xpool.tile([128, NC4 * st_sites], F32, tag="x_sb")
                    xv = x_d[b].rearrange("n d s -> (n d) s")
                    for c in range(NC4):
                        nc.sync.dma_start(
                            out=x_sb[:, c * st_sites : (c + 1) * st_sites],
                            in_=xv[c * 128 : (c + 1) * 128,
                                   st * st_sites : (st + 1) * st_sites],
                        )
                    # bf16 conversion (per chunk, ACT) and squares (DVE 2x)
                    xbf = bpool.tile([128, NC4 * st_sites], BF16, tag="xbf")
                    for c in range(NC4):
                        nc.scalar.copy(
                            xbf[:, c * st_sites : (c + 1) * st_sites],
                            x_sb[:, c * st_sites : (c + 1) * st_sites],
                        )
                    xsq = bpool.tile([128, NC4 * st_sites], BF16, tag="xsq")
                    for c in range(NC4):
                        nc.scalar.activation(
                            xsq[:, c * st_sites : (c + 1) * st_sites],
                            xbf[:, c * st_sites : (c + 1) * st_sites],
                            AF.Square,
                        )
                    out_sb = opool.tile([128, NC4 * st_sites], F32, tag="out_sb")

                    # ---- per-ST stats: 8 matmuls per tile into ps_st
                    # col layout per tile k: [k*16 + (c*2 + h2)] sums,
                    #                        [k*16 + 8 + (c*2 + h2)] sumsq
                    ps_st = stpool.tile([128, n_t * 16], F32, tag="ps_st")
                    for k in range(n_t):
                        for c in range(NC4):
                            nc.tensor.matmul(
                                ps_st[:, k * 16 + c * 2 : k * 16 + c * 2 + 2],
                                xbf[:, c * st_sites + k * TILE :
                                       c * st_sites + (k + 1) * TILE],
                                ones2[:],
                                start=True, stop=True,
                            )
                            nc.tensor.matmul(
                                ps_st[:, k * 16 + 8 + c * 2 : k * 16 + 8 + c * 2 + 2],
                                xsq[:, c * st_sites + k * TILE :
                                       c * st_sites + (k + 1) * TILE],
                                ones2[:],
                                start=True, stop=True,
                            )
                    # ---- batched stat math over [128, (k, 8)]
                    nst = n_t * 8
                    sview = ps_st[:, 0 : n_t * 16].rearrange(
                        "p (k two h) -> p k two h", k=n_t, two=2
                    )
                    mun = spool.tile([128, nst], BF16, tag="mun")      # -mu
                    musq = spool.tile([128, nst], F32, tag="musq")
                    var = spool.tile([128, nst], F32, tag="var")
                    rstd = spool.tile([128, nst], BF16, tag="rstd")
                    rstdS = spool.tile([128, nst], BF16, tag="rstdS")
                    rr = spool.tile([128, n_t * 64], BF16, tag="rr")
                    mun3 = mun[:].rearrange("p (k h) -> p k h", k=n_t)
                    nc.vector.tensor_scalar(
                        mun3, sview[:, :, 0], -1.0 / 64.0, None, op0=OP.mult
                    )
                    nc.vector.tensor_tensor(
                        musq[:].rearrange("p (k h) -> p k h", k=n_t),
                        mun3, mun3, op=OP.mult,
                    )
                    nc.vector.scalar_tensor_tensor(
                        var[:].rearrange("p (k h) -> p k h", k=n_t),
                        sview[:, :, 1], 1.0 / 64.0,
                        musq[:].rearrange("p (k h) -> p k h", k=n_t),
                        op0=OP.mult, op1=OP.subtract,
                    )
                    nc.scalar.activation(var[:], var[:], AF.Sqrt, bias=eps[:, 0:1])
                    with nc.allow_low_precision(reason="rstd in bf16"):
                        nc.vector.reciprocal(rstd[:], var[:])
                    nc.vector.tensor_scalar(rstdS[:], rstd[:], SCALE, None, op0=OP.mult)
                    # rr[p, k, i, j] = rstd_i * rstdS_j
                    nc.vector.tensor_tensor(
                        rr[:].rearrange("p (k i j) -> p k i j", k=n_t, i=8),
                        rstd[:].rearrange("p (k i) -> p k i", k=n_t)
                            .unsqueeze(3).broadcast_to([128, n_t, 8, 8]),
                        rstdS[:].rearrange("p (k j) -> p k j", k=n_t)
                            .unsqueeze(2).broadcast_to([128, n_t, 8, 8]),
                        op=OP.mult,
                    )
                    rstd3 = rstd[:].rearrange("p (k h) -> p k h", k=n_t)
                    rstdS3 = rstdS[:].rearrange("p (k h) -> p k h", k=n_t)
                    rr3 = rr[:].rearrange("p (k f) -> p k f", k=n_t)

                    # ---- phase B: per tile
                    for k in range(n_t):
                        # munT: [128,8] -> [8,128] via PE transpose
                        ps_mt = stpool.tile([128, 128], BF16, tag="ps_mt")
                        nc.tensor.transpose(
                            ps_mt[0:8, :], mun[:, k * 8 : (k + 1) * 8], idb[:]
                        )
                        mun_sb = vpool.tile([128, 128], BF16, tag="mun_sb")
                        nc.scalar.copy(mun_sb[0:8, :], ps_mt[0:8, :])

                        def xslice(c):
                            return xbf[:, c * st_sites + k * TILE :
                                          c * st_sites + (k + 1) * TILE]

                        # ---- projections: mu-row first (start=True), 4 chunks
                        ps_q = qkvpool.tile([128, 512], F32, tag="ps_q")
                        ps_k = qkvpool.tile([128, 512], F32, tag="ps_k")
                        ps_v = qkvpool.tile([128, 512], F32, tag="ps_v")
                        ps_kb = qkvpool.tile([128, 32], F32, tag="ps_kb")
                        for ps_p, ws_p in ((ps_q, wsq), (ps_k, wsk), (ps_v, wsv)):
                            nc.tensor.matmul(
                                ps_p[:], mun_sb[0:8, :], ws_p[0:8, :],
                                start=True, stop=False,
                            )
                        nc.tensor.matmul(
                            ps_kb[:, 0:32], mun_sb[0:8, :], kbmu[0:8, :],
                            start=True, stop=False,
                        )
                        for c in range(NC4):
                            xs = xslice(c)
                            for ps_p, w_p in ((ps_q, wq), (ps_k, wk), (ps_v, wv)):
                                nc.tensor.matmul(
                                    ps_p[:, c * 128 : (c + 1) * 128],
                                    xs, w_p[:],
                                    start=False, stop=(c == NC4 - 1),
                                )
                            nc.tensor.matmul(
                                ps_kb[:, c * 8 : (c + 1) * 8],
                                xs, wkf[:],
                                start=False, stop=(c == NC4 - 1),
                            )

                        # ---- PSUM -> SBUF copies
                        q_sb = vpool.tile([128, 512], BF16, tag="q_sb")
                        k_sb = vpool.tile([128, 512], BF16, tag="k_sb")
                        v_sb = vpool.tile([128, 512], BF16, tag="v_sb")
                        with tc.high_priority(offset=100):
                            nc.scalar.copy(q_sb[:], ps_q[:])
                            nc.scalar.copy(k_sb[:], ps_k[:])
                        # V stored (a, t, j) so AV products get unit inner strides
                        nc.scalar.activation(
                            v_sb[:].rearrange("p (a t j) -> p a j t", a=A, t=SD, j=8),
                            ps_v[:].rearrange("p (j a t) -> p a j t", j=8, a=A, t=SD),
                            AF.Copy,
                        )
                        # T3[p,(a,j)] = kb2 * rstdS_j ; E3 = exp(T3)
                        t3 = wpool.tile([128, 32], BF16, tag="t3")
                        t3v = t3[:].rearrange("p (a j) -> p a j", a=A)
                        nc.vector.tensor_tensor(
                            t3v,
                            ps_kb[:, 0:32].rearrange("p (j a) -> p a j", j=8),
                            rstdS3[:, k].unsqueeze(1).broadcast_to([128, A, 8]),
                            op=OP.mult,
                        )
                        e3f = wpool.tile([128, 32], BF16, tag="e3f")
                        nc.scalar.activation(e3f[:], t3[:], AF.Exp)

                        # ---- scores products [128, (a,i,j,t)] — split per a
                        prod_s = wpool.tile([128, 4096], BF16, tag="big")
                        qv = q_sb[:].rearrange("p (i a t) -> p a i t", i=8, a=A, t=SD)
                        kv = k_sb[:].rearrange("p (j a t) -> p a j t", j=8, a=A, t=SD)
                        pv5 = prod_s[:].rearrange(
                            "p (a i j t) -> p a i j t", a=A, i=8, j=8, t=SD
                        )
                        for a in range(A):
                            eng_s = nc.gpsimd if a == 3 else nc.vector
                            eng_s.tensor_tensor(
                                pv5[:, a],
                                qv[:, a].unsqueeze(2).broadcast_to([128, 8, 8, SD]),
                                kv[:, a].unsqueeze(1).broadcast_to([128, 8, 8, SD]),
                                op=OP.mult,
                            )
                        # t-tree: 16 -> 8 -> 4 -> 2 -> 1
                        t8 = wpool.tile([128, 2048], BF16, tag="t4k")
                        pv = prod_s[:].rearrange("p (r t) -> p r t", r=256, t=SD)
                        nc.vector.tensor_tensor(
                            t8[:].rearrange("p (r t) -> p r t", r=256, t=8),
                            pv[:, :, 0:8], pv[:, :, 8:16], op=OP.add,
                        )
                        t4 = wpool.tile([128, 1024], BF16, tag="t2k")
                        t8v = t8[:].rearrange("p (r t) -> p r t", r=256, t=8)
                        nc.vector.tensor_tensor(
                            t4[:].rearrange("p (r t) -> p r t", r=256, t=4),
                            t8v[:, :, 0:4], t8v[:, :, 4:8], op=OP.add,
                        )
                        t2 = wpool.tile([128, 512], BF16, tag="t1k")
                        t4v = t4[:].rearrange("p (r t) -> p r t", r=256, t=4)
                        nc.vector.tensor_tensor(
                            t2[:].rearrange("p (r t) -> p r t", r=256, t=2),
                            t4v[:, :, 0:2], t4v[:, :, 2:4], op=OP.add,
                        )
                        sraw = wpool.tile([128, 256], BF16, tag="sraw")
                        t2v = t2[:].rearrange("p (r t) -> p r t", r=256, t=2)
                        nc.vector.tensor_tensor(
                            sraw[:].rearrange("p (r t) -> p r t", r=256, t=1),
                            t2v[:, :, 0:1], t2v[:, :, 1:2], op=OP.add,
                        )
                        # e_in = sraw * rr (bcast over a) ; e = exp(e_in)
                        e_in = wpool.tile([128, 256], BF16, tag="e_in")
                        nc.vector.tensor_tensor(
                            e_in[:].rearrange("p (a i j) -> p a i j", a=A, i=8),
                            sraw[:].rearrange("p (a i j) -> p a i j", a=A, i=8),
                            rr3[:, k].rearrange("p (i j) -> p i j", i=8)
                                .unsqueeze(1).broadcast_to([128, A, 8, 8]),
                            op=OP.mult,
                        )
                        e = wpool.tile([128, 256], BF16, tag="e")
                        nc.scalar.activation(e[:], e_in[:], AF.Exp)
                        # e2 = e * E3_j (bcast over i)
                        e2 = wpool.tile([128, 256], BF16, tag="e2")
                        nc.vector.tensor_tensor(
                            e2[:].rearrange("p (a i j) -> p a i j", a=A, i=8),
                            e[:].rearrange("p (a i j) -> p a i j", a=A, i=8),
                            e3f[:].rearrange("p (a j) -> p a j", a=A)
                                .unsqueeze(2).broadcast_to([128, A, 8, 8]),
                            op=OP.mult,
                        )
                        den = wpool.tile([128, 32], F32, tag="den")
                        nc.vector.tensor_reduce(
                            den[:],
                            e2[:].rearrange("p (r j) -> p r j", r=32, j=8),
                            axis=AX.X, op=OP.add,
                        )
                        rcp = wpool.tile([128, 32], BF16, tag="rcp")
                        with nc.allow_low_precision(reason="softmax denom in bf16"):
                            nc.vector.reciprocal(rcp[:], den[:])
                        # w = rcp_(a,i) * rstd_j ; e3 = e2 * w
                        wgt = wpool.tile([128, 256], BF16, tag="wgt")
                        nc.vector.tensor_tensor(
                            wgt[:].rearrange("p (a i j) -> p a i j", a=A, i=8),
                            rcp[:].rearrange("p (a i) -> p a i", a=A)
                                .unsqueeze(3).broadcast_to([128, A, 8, 8]),
                            rstd3[:, k].unsqueeze(1).unsqueeze(1)
                                .broadcast_to([128, A, 8, 8]),
                            op=OP.mult,
                        )
                        e3 = wpool.tile([128, 256], BF16, tag="e3")
                        nc.vector.tensor_tensor(e3[:], e2[:], wgt[:], op=OP.mult)

                        # ---- AV products [128, (a,i,t,j)] — split per a
                        prod_av = wpool.tile([128, 4096], BF16, tag="big")
                        ev = e3[:].rearrange("p (a i j) -> p a i j", a=A, i=8)
                        vv = v_sb[:].rearrange("p (a t j) -> p a t j", a=A, t=SD, j=8)
                        av5 = prod_av[:].rearrange(
                            "p (a i t j) -> p a i t j", a=A, i=8, t=SD, j=8
                        )
                        for a in range(A):
                            eng = nc.vector if a <= 2 else nc.gpsimd
                            eng.tensor_tensor(
                                av5[:, a],
                                ev[:, a].unsqueeze(2).broadcast_to([128, 8, SD, 8]),
                                vv[:, a].unsqueeze(1).broadcast_to([128, 8, SD, 8]),
                                op=OP.mult,
                            )
                        # j-tree: 8 -> 4 -> 2 -> 1
                        j4 = wpool.tile([128, 2048], BF16, tag="t4k")
                        av = prod_av[:].rearrange("p (r j) -> p r j", r=512, j=8)
                        nc.vector.tensor_tensor(
                            j4[:].rearrange("p (r j) -> p r j", r=512, j=4),
                            av[:, :, 0:4], av[:, :, 4:8], op=OP.add,
                        )
                        j2 = wpool.tile([128, 1024], BF16, tag="t2k")
                        j4v = j4[:].rearrange("p (r j) -> p r j", r=512, j=4)
                        nc.vector.tensor_tensor(
                            j2[:].rearrange("p (r j) -> p r j", r=512, j=2),
                            j4v[:, :, 0:2], j4v[:, :, 2:4], op=OP.add,
                        )
                        # final level writes (i,a,t)-ordered attnout
                        attnout = vpool.tile([128, 512], BF16, tag="attnout")
                        j2v = j2[:].rearrange(
                            "p (a i t j) -> p a i t j", a=A, i=8, t=SD, j=2
                        )
                        nc.vector.tensor_tensor(
                            attnout[:].rearrange(
                                "p (i a t) -> p a i t", i=8, a=A, t=SD
                            ),
                            j2v[:, :, :, :, 0],
                            j2v[:, :, :, :, 1],
                            op=OP.add,
                        )
                        # ---- out projection
                        ps_aoT = oppool.tile([128, 512], BF16, tag="ps_aoT")
                        for c in range(NC4):
                            nc.tensor.transpose(
                                ps_aoT[:, c * 128 : (c + 1) * 128],
                                attnout[:, c * 128 : (c + 1) * 128],
                                idb[:],
                            )
                        ao_sb = vpool.tile([128, 512], BF16, tag="ao_sb")
                        nc.scalar.copy(ao_sb[:], ps_aoT[:])
                        ps_o = oppool.tile([128, 512], F32, tag="ps_o")
                        # residual first (c0 start=True clears bank)
                        for c in range(NC4):
                            nc.tensor.matmul(
                                ps_o[:, c * 128 : (c + 1) * 128],
                                idb[:], xslice(c),
                                start=(c == 0), stop=False,
                            )
                        for c in range(NC4):
                            nc.tensor.matmul(
                                ps_o[:, c * 128 : (c + 1) * 128],
                                wo[:],
                                ao_sb[:, c * 128 : (c + 1) * 128],
                                start=False, stop=(c == NC4 - 1),
                            )
                        # + bo, PSUM -> out_sb
                        ov = out_sb[:].rearrange(
                            "p (c s) -> p c s", c=NC4, s=st_sites
                        )[:, :, k * TILE : (k + 1) * TILE]
                        nc.scalar.activation(
                            ov,
                            ps_o[:].rearrange("p (c s) -> p c s", c=NC4, s=TILE),
                            AF.Identity,
                            bias=bo[:, 0:1],
                        )
                    # ---- store super-tile
                    ovd = out_d[b].rearrange("n d s -> (n d) s")
                    for c in range(NC4):
                        nc.sync.dma_start(
                            out=ovd[c * 128 : (c + 1) * 128,
                                    st * st_sites : (st + 1) * st_sites],
                            in_=out_sb[:, c * st_sites : (c + 1) * st_sites],
                        )
    return nc


def _prep_consts(Wq, bq, Wk, bk, Wv, bv, Wo, bo, ln_g, ln_b):
    f32 = np.float32
    bf = ml_dtypes.bfloat16
    Wq, bq, Wk, bk, Wv, bv, Wo, bo, ln_g, ln_b = [
        np.asarray(t, f32) for t in (Wq, bq, Wk, bk, Wv, bv, Wo, bo, ln_g, ln_b)
    ]
    # fold LN gain into projections; LN beta into per-head biases
    Wq_p, Wk_p, Wv_p = Wq * ln_g[None, :], Wk * ln_g[None, :], Wv * ln_g[None, :]
    BQ = bq + Wq @ ln_b
    BK = bk + Wk @ ln_b  # noqa: F841  (enters only via cc, which cancels)
    BV = bv + Wv @ ln_b
    # kb-fold: kb2[s,(j,a)] = xc_j . wkf[:,a], wkf[d,a] = sum_t Wk_p[(a,t),d]*BQ[(a,t)]
    wkf = np.zeros((64, A), f32)
    for a in range(A):
        for t in range(SD):
            wkf[:, a] += Wk_p[a * SD + t, :] * BQ[a * SD + t]
    bo_p = bo + Wo @ BV  # V-bias folded through out-projection

    def blockdiag(W):
        m = np.zeros((128, 128), f32)
        m[:64, :64] = W.T
        m[64:, 64:] = W.T
        return m.astype(bf)

    wkf_bd = np.zeros((128, 8), f32)
    wkf_bd[:64, 0:4] = wkf
    wkf_bd[64:, 4:8] = wkf

    # mu-row movings: row h, col (h',o): delta * wsum[o]
    def wsum_rows(Wp):
        ws = Wp.sum(axis=1)  # [64] per output col o
        m = np.zeros((8, 512), f32)
        for h in range(8):
            m[h, h * 64 : (h + 1) * 64] = ws
        return m.astype(bf)

    kbmu = np.zeros((8, 32), f32)
    wkfsum = wkf.sum(axis=0)  # [A]
    for j in range(8):
        kbmu[j, j * 4 : (j + 1) * 4] = wkfsum

    ones2 = np.zeros((128, 2), f32)
    ones2[:64, 0] = 1.0
    ones2[64:, 1] = 1.0

    consts = {
        "wq_bd": blockdiag(Wq_p),
        "wk_bd": blockdiag(Wk_p),
        "wv_bd": blockdiag(Wv_p),
        "wo_bd": blockdiag(Wo),
        "wkf_bd": wkf_bd.astype(bf),
        "wsum_q": wsum_rows(Wq_p),
        "wsum_k": wsum_rows(Wk_p),
        "wsum_v": wsum_rows(Wv_p),
        "kbmu": kbmu.astype(bf),
        "ones2": ones2.astype(bf),
        "ident_bf": np.eye(128).astype(bf),
        "bo_col": np.tile(bo_p, 2)[:, None].astype(f32),
        "eps_col": np.full((128, 1), LN_EPS, f32),
    }
    return consts


def kernel(x, Wq, bq, Wk, bk, Wv, bv, Wo, bo, ln_g, ln_b):
    x = np.asarray(x, np.float32)
    B, n, d, H, W = x.shape
    S = H * W
    bpc = B // N_CORES
    consts = _prep_consts(Wq, bq, Wk, bk, Wv, bv, Wo, bo, ln_g, ln_b)

    nc = build_nc(n_b=bpc, s_total=S, st_sites=1024 if S % 1024 == 0 else S)
    xr = x.reshape(B, n, d, S)
    in_maps = []
    for c in range(N_CORES):
        m = dict(consts)
        m["x"] = np.ascontiguousarray(xr[c * bpc : (c + 1) * bpc])
        in_maps.append(m)
    res = run_bass_kernel_spmd(nc, in_maps, core_ids=list(range(N_CORES)))
    outs = [res.results[i]["out"] for i in range(N_CORES)]
    out = np.concatenate(outs, axis=0).reshape(B, n, d, H, W)
    return out.astype(np.float32)


# revision 60
# speedup vs baseline: 1.0073x; 1.0048x over previous
"""Trainium2 Bass kernel for CrossHeadMultiHeadAttention (v2).

Computation (per batch b, spatial site s): LN over d=64 per head (8 heads),
torch-Linear Q/K/V, cross-head attention within 4 groups of 16 dims,
out-projection, residual.

v2 strategy (data-parallel over batch, 16 -> 8 cores x 2):
  - x stays in d-on-partition layout [(head,d), sites]; NO transposes for LN.
  - LN stats via PE matmuls (ones-moving, N=2); mean-centering enters the
    Q/K/V projections as an extra "-mu" stationary row (mu-row trick).
  - rstd and biases never touch Q/K/V tensors: scores_true factorizes as
      exp(S*scores) = exp(rr_ij * QKc) * [i-terms cancel in softmax]
                      * exp(rstdS_j * kb2)_j
    with rr = rstd_i*rstd_j*SCALE, kb2 = sum_t Kc*bq' (extra matmul cols),
    V-bias folded into the out-projection bias host-side.
  - attention core (per-site 8x8x(4 groups) QK^T / AV) on DVE/Pool as
    broadcast-product tensors + halving-tree reductions, bf16, 2x mode.
  - out-projection via block-diag Wo matmul; residual via identity matmul
    of bf16 x; out bias rides the ACT PSUM->SBUF copy.
"""

import json

import numpy as np
import ml_dtypes

import concourse.bass as bass
import concourse.mybir as mybir
from concourse.tile import TileContext
from concourse.bass_utils import run_bass_kernel_spmd
import concourse.bass_utils as _bass_utils
import concourse.bass2jax as _bass2jax
import bass_rust

F32 = mybir.dt.float32
BF16 = mybir.dt.bfloat16
AX = mybir.AxisListType
OP = mybir.AluOpType
AF = mybir.ActivationFunctionType

N_HEADS = 8
D = 64
A = 4          # attention groups
SD = 16        # sub dim per group
SCALE = SD ** -0.5
LN_EPS = 1e-5
N_CORES = 8

_PATCHED = False

# this walrus build accepts fewer sync-wait commands per instruction than
# bass emits; hoist the excess onto EventSemaphore carriers just before.
_WAIT_CAPS = {"Drain": 0, "Nop": 0, "EventSemaphore": 2}
_DEFAULT_WAIT_CAP = 1


def _fix_bir_waits(bir: bytes) -> bytes:
    j = json.loads(bir)
    ctr = 0
    changed = False
    for f in j.get("functions", []):
        for blk in f.get("blocks", []):
            out = []
            for ins in blk.get("instructions", []):
                si = ins.get("sync_info") or {}
                ow = si.get("on_wait") or []
                cap = _WAIT_CAPS.get(ins.get("opcode"), _DEFAULT_WAIT_CAP)
                if len(ow) > cap:
                    changed = True
                    n_keep = cap
                    excess, keep = ow[: len(ow) - n_keep], ow[len(ow) - n_keep :]
                    for i in range(0, len(excess), 2):
                        ctr += 1
                        chunk = excess[i : i + 2]
                        w0 = chunk[0]
                        out.append({
                            "debug": ins.get("debug", 0),
                            "engine": ins.get("engine"),
                            "ins": [],
                            "outs": [],
                            "name": f"waitfix_{ctr}",
                            "opcode": "EventSemaphore",
                            "sync_info": {
                                "on_update": [{
                                    "ant_name": w0["ant_name"],
                                    "id": w0["id"],
                                    "sync_type": "semaphore",
                                    "update_mode": "sem-add-imm",
                                    "update_value": 0,
                                }],
                                "on_wait": chunk,
                            },
                        })
                    si = dict(si)
                    si["on_wait"] = keep
                    ins = dict(ins)
                    ins["sync_info"] = si
                out.append(ins)
            blk["instructions"] = out
    if not changed:
        return bir
    return json.dumps(j).encode()


_orig_compile_bir_kernel = _bass_utils.compile_bir_kernel


def _compile_bir_kernel_fixed(bir_json, tmpdir, neff_name="file.neff"):
    if isinstance(bir_json, str):
        bir_json = bir_json.encode()
    return _orig_compile_bir_kernel(_fix_bir_waits(bir_json), tmpdir, neff_name=neff_name)


def _patch_tile_drain():
    """walrus here rejects >2 sem waits on the Tile tail-drain; spread the
    waits over EventSemaphore carriers (<=2 waits each) instead."""
    global _PATCHED
    if _PATCHED:
        return
    _PATCHED = True
    _bass_utils.compile_bir_kernel = _compile_bir_kernel_fixed
    _bass2jax.compile_bir_kernel = _compile_bir_kernel_fixed
    ScopedClock = bass_rust.ScopedClock

    def patched(self, tick_clock, wait_clock):
        nc = self.nc
        sems = list(self.sems.allocated().values())
        if sems:
            carrier = nc.sync.sem_inc(sems[0], 0)
            wait_clock.add_sem_waits(
                carrier.ins, ScopedClock({None: tick_clock.global_clock})
            )
            si = carrier.ins.sync_info
            waits = list(si.on_wait) if si else []
            if len(waits) > 2:
                carrier.ins.sync_info = bass_rust.SyncInfo(
                    on_wait=waits[:2], on_update=list(si.on_update)
                )
                for i in range(2, len(waits), 2):
                    c2 = nc.sync.sem_inc(sems[0], 0)
                    si2 = c2.ins.sync_info
                    c2.ins.sync_info = bass_rust.SyncInfo(
                        on_wait=waits[i : i + 2],
                        on_update=list(si2.on_update) if si2 else [],
                    )
        nc.sync.drain()
        nc.all_engine_barrier()
        popped = nc._tile_sem_poison_stack.pop()
        assert popped is self._sem_poison
        nc.clear_and_free_semaphores(sems)
        nc.all_engine_barrier()

    TileContext._drain_and_barrier = patched


def build_nc(n_b: int, s_total: int, st_sites: int):
    """Build the per-core SPMD program.

    n_b: batches per core; s_total: sites per batch (H*W);
    st_sites: sites per super-tile (DMA granularity), multiple of 128.
    """
    _patch_tile_drain()
    nc = bass.Bass()
    TILE = 128
    n_st = s_total // st_sites
    n_t = st_sites // TILE
    NC4 = 4  # head-pair chunks

    x_d = nc.dram_tensor("x", [n_b, N_HEADS, D, s_total], F32, kind="ExternalInput")
    wq_d = nc.dram_tensor("wq_bd", [128, 128], BF16, kind="ExternalInput")
    wk_d = nc.dram_tensor("wk_bd", [128, 128], BF16, kind="ExternalInput")
    wv_d = nc.dram_tensor("wv_bd", [128, 128], BF16, kind="ExternalInput")
    wo_d = nc.dram_tensor("wo_bd", [128, 128], BF16, kind="ExternalInput")
    wkf_d = nc.dram_tensor("wkf_bd", [128, 8], BF16, kind="ExternalInput")
    wsq_d = nc.dram_tensor("wsum_q", [8, 512], BF16, kind="ExternalInput")
    wsk_d = nc.dram_tensor("wsum_k", [8, 512], BF16, kind="ExternalInput")
    wsv_d = nc.dram_tensor("wsum_v", [8, 512], BF16, kind="ExternalInput")
    kbmu_d = nc.dram_tensor("kbmu", [8, 32], BF16, kind="ExternalInput")
    ones2_d = nc.dram_tensor("ones2", [128, 2], BF16, kind="ExternalInput")
    idb_d = nc.dram_tensor("ident_bf", [128, 128], BF16, kind="ExternalInput")
    bo_d = nc.dram_tensor("bo_col", [128, 1], F32, kind="ExternalInput")
    eps_d = nc.dram_tensor("eps_col", [128, 1], F32, kind="ExternalInput")
    out_d = nc.dram_tensor("out", [n_b, N_HEADS, D, s_total], F32, kind="ExternalOutput")

    with TileContext(nc) as tc:
        with (
            tc.tile_pool(name="consts", bufs=1) as cpool,
            tc.tile_pool(name="xio", bufs=2) as xpool,
            tc.tile_pool(name="xbfp", bufs=2) as bpool,
            tc.tile_pool(name="oio", bufs=2) as opool,
            tc.tile_pool(name="work", bufs=4) as wpool,
            tc.tile_pool(name="vecs", bufs=4) as vpool,
            tc.tile_pool(name="stats", bufs=2) as spool,
            tc.tile_pool(name="psst", bufs=1, space="PSUM") as stpool,
            tc.tile_pool(name="psqkv", bufs=1, space="PSUM") as qkvpool,
            tc.tile_pool(name="pso", bufs=1, space="PSUM") as oppool,
        ):
            # ---- constants into SBUF
            def cload(dram, shape, dtype, tag, rows=None):
                t = cpool.tile(shape, dtype, tag=tag)
                if rows is None:
                    nc.sync.dma_start(out=t[:], in_=dram[:])
                else:
                    nc.sync.dma_start(out=t[0:rows, :], in_=dram[:])
                return t

            wq = cload(wq_d, [128, 128], BF16, "wq")
            wk = cload(wk_d, [128, 128], BF16, "wk")
            wv = cload(wv_d, [128, 128], BF16, "wv")
            wo = cload(wo_d, [128, 128], BF16, "wo")
            wkf = cload(wkf_d, [128, 8], BF16, "wkf")
            wsq = cload(wsq_d, [128, 512], BF16, "wsq", rows=8)
            wsk = cload(wsk_d, [128, 512], BF16, "wsk", rows=8)
            wsv = cload(wsv_d, [128, 512], BF16, "wsv", rows=8)
            kbmu = cload(kbmu_d, [128, 32], BF16, "kbmu", rows=8)
            ones2 = cload(ones2_d, [128, 2], BF16, "ones2")
            idb = cload(idb_d, [128, 128], BF16, "idb")
            bo = cload(bo_d, [128, 1], F32, "bo")
            eps = cload(eps_d, [128, 1], F32, "eps")

            for b in range(n_b):
                for st in range(n_st):
                    # ---- load super-tile: 4 chunks of [128=(2n,64d), st_sites]
                    x_sb = xpool.tile([128, NC4 * st_sites], F32, tag="x_sb")
                    xv = x_d[b].rearrange("n d s -> (n d) s")
                    for c in range(NC4):
                        nc.sync.dma_start(
                            out=x_sb[:, c * st_sites : (c + 1) * st_sites],
                            in_=xv[c * 128 : (c + 1) * 128,
                                   st * st_sites : (st + 1) * st_sites],
                        )
                    # bf16 conversion (per chunk, ACT) and squares (DVE 2x)
                    xbf = bpool.tile([128, NC4 * st_sites], BF16, tag="xbf")
                    for c in range(NC4):
                        nc.scalar.copy(
                            xbf[:, c * st_sites : (c + 1) * st_sites],
                            x_sb[:, c * st_sites : (c + 1) * st_sites],
                        )
                    xsq = bpool.tile([128, NC4 * st_sites], BF16, tag="xsq")
                    for c in range(NC4):
                        nc.scalar.activation(
                            xsq[:, c * st_sites : (c + 1) * st_sites],
                            xbf[:, c * st_sites : (c + 1) * st_sites],
                            AF.Square,
                        )
                    out_sb = opool.tile([128, NC4 * st_sites], F32, tag="out_sb")

                    # ---- per-ST stats: 8 matmuls per tile into ps_st
                    # col layout per tile k: [k*16 + (c*2 + h2)] sums,
                    #                        [k*16 + 8 + (c*2 + h2)] sumsq
                    ps_st = stpool.tile([128, n_t * 16], F32, tag="ps_st")
                    for k in range(n_t):
                        for c in range(NC4):
                            nc.tensor.matmul(
                                ps_st[:, k * 16 + c * 2 : k * 16 + c * 2 + 2],
                                xbf[:, c * st_sites + k * TILE :
                                       c * st_sites + (k + 1) * TILE],
                                ones2[:],
                                start=True, stop=True,
                            )
                            nc.tensor.matmul(
                                ps_st[:, k * 16 + 8 + c * 2 : k * 16 + 8 + c * 2 + 2],
                                xsq[:, c * st_sites + k * TILE :
                                       c * st_sites + (k + 1) * TILE],
                                ones2[:],
                                start=True, stop=True,
                            )
                    # ---- batched stat math over [128, (k, 8)]
                    nst = n_t * 8
                    sview = ps_st[:, 0 : n_t * 16].rearrange(
                        "p (k two h) -> p k two h", k=n_t, two=2
                    )
                    mun = spool.tile([128, nst], BF16, tag="mun")      # -mu
                    musq = spool.tile([128, nst], F32, tag="musq")
                    var = spool.tile([128, nst], F32, tag="var")
                    rstd = spool.tile([128, nst], BF16, tag="rstd")
                    rstdS = spool.tile([128, nst], BF16, tag="rstdS")
                    rr = spool.tile([128, n_t * 64], BF16, tag="rr")
                    mun3 = mun[:].rearrange("p (k h) -> p k h", k=n_t)
                    nc.vector.tensor_scalar(
                        mun3, sview[:, :, 0], -1.0 / 64.0, None, op0=OP.mult
                    )
                    nc.vector.tensor_tensor(
                        musq[:].rearrange("p (k h) -> p k h", k=n_t),
                        mun3, mun3, op=OP.mult,
                    )
                    nc.vector.scalar_tensor_tensor(
                        var[:].rearrange("p (k h) -> p k h", k=n_t),
                        sview[:, :, 1], 1.0 / 64.0,
                        musq[:].rearrange("p (k h) -> p k h", k=n_t),
                        op0=OP.mult, op1=OP.subtract,
                    )
                    nc.scalar.activation(var[:], var[:], AF.Sqrt, bias=eps[:, 0:1])
                    with nc.allow_low_precision(reason="rstd in bf16"):
                        nc.vector.reciprocal(rstd[:], var[:])
                    nc.vector.tensor_scalar(rstdS[:], rstd[:], SCALE, None, op0=OP.mult)
                    # rr[p, k, i, j] = rstd_i * rstdS_j
                    nc.vector.tensor_tensor(
                        rr[:].rearrange("p (k i j) -> p k i j", k=n_t, i=8),
                        rstd[:].rearrange("p (k i) -> p k i", k=n_t)
                            .unsqueeze(3).broadcast_to([128, n_t, 8, 8]),
                        rstdS[:].rearrange("p (k j) -> p k j", k=n_t)
                            .unsqueeze(2).broadcast_to([128, n_t, 8, 8]),
                        op=OP.mult,
                    )
                    rstd3 = rstd[:].rearrange("p (k h) -> p k h", k=n_t)
                    rstdS3 = rstdS[:].rearrange("p (k h) -> p k h", k=n_t)
                    rr3 = rr[:].rearrange("p (k f) -> p k f", k=n_t)

                    # ---- phase B: per tile
                    for k in range(n_t):
                        # munT: [128,8] -> [8,128] via PE transpose
                        ps_mt = stpool.tile([128, 128], BF16, tag="ps_mt")
                        nc.tensor.transpose(
                            ps_mt[0:8, :], mun[:, k * 8 : (k + 1) * 8], idb[:]
                        )
                        mun_sb = vpool.tile([128, 128], BF16, tag="mun_sb")
                        nc.scalar.copy(mun_sb[0:8, :], ps_mt[0:8, :])

                        def xslice(c):
                            return xbf[:, c * st_sites + k * TILE :
                                          c * st_sites + (k + 1) * TILE]

                        # ---- projections: mu-row first (start=True), 4 chunks
                        ps_q = qkvpool.tile([128, 512], F32, tag="ps_q")
                        ps_k = qkvpool.tile([128, 512], F32, tag="ps_k")
                        ps_v = qkvpool.tile([128, 512], F32, tag="ps_v")
                        ps_kb = qkvpool.tile([128, 32], F32, tag="ps_kb")
                        for ps_p, ws_p in ((ps_q, wsq), (ps_k, wsk), (ps_v, wsv)):
                            nc.tensor.matmul(
                                ps_p[:], mun_sb[0:8, :], ws_p[0:8, :],
                                start=True, stop=False,
                            )
                        nc.tensor.matmul(
                            ps_kb[:, 0:32], mun_sb[0:8, :], kbmu[0:8, :],
                            start=True, stop=False,
                        )
                        for c in range(NC4):
                            xs = xslice(c)
                            for ps_p, w_p in ((ps_q, wq), (ps_k, wk), (ps_v, wv)):
                                nc.tensor.matmul(
                                    ps_p[:, c * 128 : (c + 1) * 128],
                                    xs, w_p[:],
                                    start=False, stop=(c == NC4 - 1),
                                )
                            nc.tensor.matmul(
                                ps_kb[:, c * 8 : (c + 1) * 8],
                                xs, wkf[:],
                                start=False, stop=(c == NC4 - 1),
                            )

                        # ---- PSUM -> SBUF copies
                        q_sb = vpool.tile([128, 512], BF16, tag="q_sb")
                        k_sb = vpool.tile([128, 512], BF16, tag="k_sb")
                        v_sb = vpool.tile([128, 512], BF16, tag="v_sb")
                        with tc.high_priority(offset=100):
                            nc.scalar.copy(q_sb[:], ps_q[:])
                            nc.scalar.copy(k_sb[:], ps_k[:])
                        # V stored (a, t, j) so AV products get unit inner strides
                        nc.scalar.activation(
                            v_sb[:].rearrange("p (a t j) -> p a j t", a=A, t=SD, j=8),
                            ps_v[:].rearrange("p (j a t) -> p a j t", j=8, a=A, t=SD),
                            AF.Copy,
                        )
                        # T3[p,(a,j)] = kb2 * rstdS_j ; E3 = exp(T3)
                        t3 = wpool.tile([128, 32], BF16, tag="t3")
                        t3v = t3[:].rearrange("p (a j) -> p a j", a=A)
                        nc.vector.tensor_tensor(
                            t3v,
                            ps_kb[:, 0:32].rearrange("p (j a) -> p a j", j=8),
                            rstdS3[:, k].unsqueeze(1).broadcast_to([128, A, 8]),
                            op=OP.mult,
                        )
                        e3f = wpool.tile([128, 32], BF16, tag="e3f")
                        nc.scalar.activation(e3f[:], t3[:], AF.Exp)

                        # ---- scores products [128, (a,i,j,t)] — split per a
                        prod_s = wpool.tile([128, 4096], BF16, tag="big")
                        qv = q_sb[:].rearrange("p (i a t) -> p a i t", i=8, a=A, t=SD)
                        kv = k_sb[:].rearrange("p (j a t) -> p a j t", j=8, a=A, t=SD)
                        pv5 = prod_s[:].rearrange(
                            "p (a i j t) -> p a i j t", a=A, i=8, j=8, t=SD
                        )
                        for a in range(A):
                            eng_s = nc.gpsimd if a == 3 else nc.vector
                            eng_s.tensor_tensor(
                                pv5[:, a],
                                qv[:, a].unsqueeze(2).broadcast_to([128, 8, 8, SD]),
                                kv[:, a].unsqueeze(1).broadcast_to([128, 8, 8, SD]),
                                op=OP.mult,
                            )
                        # t-tree: 16 -> 8 -> 4 -> 2 -> 1
                        t8 = wpool.tile([128, 2048], BF16, tag="t4k")
                        pv = prod_s[:].rearrange("p (r t) -> p r t", r=256, t=SD)
                        nc.vector.tensor_tensor(
                            t8[:].rearrange("p (r t) -> p r t", r=256, t=8),
                            pv[:, :, 0:8], pv[:, :, 8:16], op=OP.add,
                        )
                        t4 = wpool.tile([128, 1024], BF16, tag="t2k")
                        t8v = t8[:].rearrange("p (r t) -> p r t", r=256, t=8)
                        nc.vector.tensor_tensor(
                            t4[:].rearrange("p (r t) -> p r t", r=256, t=4),
                            t8v[:, :, 0:4], t8v[:, :, 4:8], op=OP.add,
                        )
                        t2 = wpool.tile([128, 512], BF16, tag="t1k")
                        t4v = t4[:].rearrange("p (r t) -> p r t", r=256, t=4)
                        nc.vector.tensor_tensor(
                            t2[:].rearrange("p (r t) -> p r t", r=256, t=2),
                            t4v[:, :, 0:2], t4v[:, :, 2:4], op=OP.add,
                        )
                        sraw = wpool.tile([128, 256], BF16, tag="sraw")
                        t2v = t2[:].rearrange("p (r t) -> p r t", r=256, t=2)
                        nc.vector.tensor_tensor(
                            sraw[:].rearrange("p (r t) -> p r t", r=256, t=1),
                            t2v[:, :, 0:1], t2v[:, :, 1:2], op=OP.add,
                        )
                        # e_in = sraw * rr (bcast over a) ; e = exp(e_in)
                        e_in = wpool.tile([128, 256], BF16, tag="e_in")
                        nc.vector.tensor_tensor(
                            e_in[:].rearrange("p (a i j) -> p a i j", a=A, i=8),
                            sraw[:].rearrange("p (a i j) -> p a i j", a=A, i=8),
                            rr3[:, k].rearrange("p (i j) -> p i j", i=8)
                                .unsqueeze(1).broadcast_to([128, A, 8, 8]),
                            op=OP.mult,
                        )
                        e = wpool.tile([128, 256], BF16, tag="e")
                        nc.scalar.activation(e[:], e_in[:], AF.Exp)
                        # e2 = e * E3_j (bcast over i)
                        e2 = wpool.tile([128, 256], BF16, tag="e2")
                        nc.vector.tensor_tensor(
                            e2[:].rearrange("p (a i j) -> p a i j", a=A, i=8),
                            e[:].rearrange("p (a i j) -> p a i j", a=A, i=8),
                            e3f[:].rearrange("p (a j) -> p a j", a=A)
                                .unsqueeze(2).broadcast_to([128, A, 8, 8]),
                            op=OP.mult,
                        )
                        den = wpool.tile([128, 32], F32, tag="den")
                        nc.vector.tensor_reduce(
                            den[:],
                            e2[:].rearrange("p (r j) -> p r j", r=32, j=8),
                            axis=AX.X, op=OP.add,
                        )
                        rcp = wpool.tile([128, 32], BF16, tag="rcp")
                        with nc.allow_low_precision(reason="softmax denom in bf16"):
                            nc.vector.reciprocal(rcp[:], den[:])
                        # w = rcp_(a,i) * rstd_j ; e3 = e2 * w
                        wgt = wpool.tile([128, 256], BF16, tag="wgt")
                        nc.vector.tensor_tensor(
                            wgt[:].rearrange("p (a i j) -> p a i j", a=A, i=8),
                            rcp[:].rearrange("p (a i) -> p a i", a=A)
                                .unsqueeze(3).broadcast_to([128, A, 8, 8]),
                            rstd3[:, k].unsqueeze(1).unsqueeze(1)
                                .broadcast_to([128, A, 8, 8]),
                            op=OP.mult,
                        )
                        e3 = wpool.tile([128, 256], BF16, tag="e3")
                        nc.vector.tensor_tensor(e3[:], e2[:], wgt[:], op=OP.mult)

                        # ---- AV products [128, (a,i,t,j)] — split per a
                        prod_av = wpool.tile([128, 4096], BF16, tag="big")
                        ev = e3[:].rearrange("p (a i j) -> p a i j", a=A, i=8)
                        vv = v_sb[:].rearrange("p (a t j) -> p a t j", a=A, t=SD, j=8)
                        av5 = prod_av[:].rearrange(
                            "p (a i t j) -> p a i t j", a=A, i=8, t=SD, j=8
                        )
                        for a in range(A):
                            eng = nc.vector if a <= 2 else nc.gpsimd
                            eng.tensor_tensor(
                                av5[:, a],
                                ev[:, a].unsqueeze(2).broadcast_to([128, 8, SD, 8]),
                                vv[:, a].unsqueeze(1).broadcast_to([128, 8, SD, 8]),
                                op=OP.mult,
                            )
                        # j-tree: 8 -> 4 -> 2 -> 1
                        j4 = wpool.tile([128, 2048], BF16, tag="t4k")
                        av = prod_av[:].rearrange("p (r j) -> p r j", r=512, j=8)
                        nc.vector.tensor_tensor(
                            j4[:].rearrange("p (r j) -> p r j", r=512, j=4),
                            av[:, :, 0:4], av[:, :, 4:8], op=OP.add,
                        )
                        j2 = wpool.tile([128, 1024], BF16, tag="t2k")
                        j4v = j4[:].rearrange("p (r j) -> p r j", r=512, j=4)
                        nc.vector.tensor_tensor(
                            j2[:].rearrange("p (r j) -> p r j", r=512, j=2),
                            j4v[:, :, 0:2], j4v[:, :, 2:4], op=OP.add,
                        )
                        # final level writes (i,a,t)-ordered attnout
                        attnout = vpool.tile([128, 512], BF16, tag="attnout")
                        j2v = j2[:].rearrange(
                            "p (a i t j) -> p a i t j", a=A, i=8, t=SD, j=2
                        )
                        nc.vector.tensor_tensor(
                            attnout[:].rearrange(
                                "p (i a t) -> p a i t", i=8, a=A, t=SD
                            ),
                            j2v[:, :, :, :, 0],
                            j2v[:, :, :, :, 1],
                            op=OP.add,
                        )
                        # ---- out projection
                        ps_aoT = oppool.tile([128, 512], BF16, tag="ps_aoT")
                        for c in range(NC4):
                            nc.tensor.transpose(
                                ps_aoT[:, c * 128 : (c + 1) * 128],
                                attnout[:, c * 128 : (c + 1) * 128],
                                idb[:],
                            )
                        ao_sb = vpool.tile([128, 512], BF16, tag="ao_sb")
                        nc.scalar.copy(ao_sb[:], ps_aoT[:])
                        ps_o = oppool.tile([128, 512], F32, tag="ps_o")
                        # residual first (c0 start=True clears bank)
                        for c in range(NC4):
                            nc.tensor.matmul(
                                ps_o[:, c * 128 : (c + 1) * 128],
                                idb[:], xslice(c),
                                start=(c == 0), stop=False,
                            )
                        for c in range(NC4):
                            nc.tensor.matmul(
                                ps_o[:, c * 128 : (c + 1) * 128],
                                wo[:],
                                ao_sb[:, c * 128 : (c + 1) * 128],
                                start=False, stop=(c == NC4 - 1),
                            )
                        # + bo, PSUM -> out_sb
                        ov = out_sb[:].rearrange(
                            "p (c s) -> p c s", c=NC4, s=st_sites
                        )[:, :, k * TILE : (k + 1) * TILE]
                        nc.scalar.activation(
                            ov,
                            ps_o[:].rearrange("p (c s) -> p c s", c=NC4, s=TILE),
                            AF.Identity,
                            bias=bo[:, 0:1],
                        )
                    # ---- store super-tile (two half-ST flushes to cut tail)
                    ovd = out_d[b].rearrange("n d s -> (n d) s")
                    half = st_sites // 8
                    for hh in range(8):
                        for c in range(NC4):
                            nc.sync.dma_start(
                                out=ovd[c * 128 : (c + 1) * 128,
                                        st * st_sites + hh * half :
                                        st * st_sites + (hh + 1) * half],
                                in_=out_sb[:, c * st_sites + hh * half :
                                              c * st_sites + (hh + 1) * half],
                            )
    return nc


def _prep_consts(Wq, bq, Wk, bk, Wv, bv, Wo, bo, ln_g, ln_b):
    f32 = np.float32
    bf = ml_dtypes.bfloat16
    Wq, bq, Wk, bk, Wv, bv, Wo, bo, ln_g, ln_b = [
        np.asarray(t, f32) for t in (Wq, bq, Wk, bk, Wv, bv, Wo, bo, ln_g, ln_b)
    ]
    # fold LN gain into projections; LN beta into per-head biases
    Wq_p, Wk_p, Wv_p = Wq * ln_g[None, :], Wk * ln_g[None, :], Wv * ln_g[None, :]
    BQ = bq + Wq @ ln_b
    BK = bk + Wk @ ln_b  # noqa: F841  (enters only via cc, which cancels)
    BV = bv + Wv @ ln_b
    # kb-fold: kb2[s,(j,a)] = xc_j . wkf[:,a], wkf[d,a] = sum_t Wk_p[(a,t),d]*BQ[(a,t)]
    wkf = np.zeros((64, A), f32)
    for a in range(A):
        for t in range(SD):
            wkf[:, a] += Wk_p[a * SD + t, :] * BQ[a * SD + t]
    bo_p = bo + Wo @ BV  # V-bias folded through out-projection

    def blockdiag(W):
        m = np.zeros((128, 128), f32)
        m[:64, :64] = W.T
        m[64:, 64:] = W.T
        return m.astype(bf)

    wkf_bd = np.zeros((128, 8), f32)
    wkf_bd[:64, 0:4] = wkf
    wkf_bd[64:, 4:8] = wkf

    # mu-row movings: row h, col (h',o): delta * wsum[o]
    def wsum_rows(Wp):
        ws = Wp.sum(axis=1)  # [64] per output col o
        m = np.zeros((8, 512), f32)
        for h in range(8):
            m[h, h * 64 : (h + 1) * 64] = ws
        return m.astype(bf)

    kbmu = np.zeros((8, 32), f32)
    wkfsum = wkf.sum(axis=0)  # [A]
    for j in range(8):
        kbmu[j, j * 4 : (j + 1) * 4] = wkfsum

    ones2 = np.zeros((128, 2), f32)
    ones2[:64, 0] = 1.0
    ones2[64:, 1] = 1.0

    consts = {
        "wq_bd": blockdiag(Wq_p),
        "wk_bd": blockdiag(Wk_p),
        "wv_bd": blockdiag(Wv_p),
        "wo_bd": blockdiag(Wo),
        "wkf_bd": wkf_bd.astype(bf),
        "wsum_q": wsum_rows(Wq_p),
        "wsum_k": wsum_rows(Wk_p),
        "wsum_v": wsum_rows(Wv_p),
        "kbmu": kbmu.astype(bf),
        "ones2": ones2.astype(bf),
        "ident_bf": np.eye(128).astype(bf),
        "bo_col": np.tile(bo_p, 2)[:, None].astype(f32),
        "eps_col": np.full((128, 1), LN_EPS, f32),
    }
    return consts


def kernel(x, Wq, bq, Wk, bk, Wv, bv, Wo, bo, ln_g, ln_b):
    x = np.asarray(x, np.float32)
    B, n, d, H, W = x.shape
    S = H * W
    bpc = B // N_CORES
    consts = _prep_consts(Wq, bq, Wk, bk, Wv, bv, Wo, bo, ln_g, ln_b)

    nc = build_nc(n_b=bpc, s_total=S, st_sites=1024 if S % 1024 == 0 else S)
    xr = x.reshape(B, n, d, S)
    in_maps = []
    for c in range(N_CORES):
        m = dict(consts)
        m["x"] = np.ascontiguousarray(xr[c * bpc : (c + 1) * bpc])
        in_maps.append(m)
    res = run_bass_kernel_spmd(nc, in_maps, core_ids=list(range(N_CORES)))
    outs = [res.results[i]["out"] for i in range(N_CORES)]
    out = np.concatenate(outs, axis=0).reshape(B, n, d, H, W)
    return out.astype(np.float32)
